# revision 6
# baseline (speedup 1.0000x reference)
"""Sparse-attention layer on 8 TRN2 NeuronCores (data-parallel over batch).

Reference computation (per batch b):
    q = states @ Wq; k = key @ Wk; v = key @ Wv            [T, H, A]
    alpha[h,q,k] = q.k + bs[q,k]*ksum[k,h]                 (bs = sparse edge bias scatter)
    alpha = alpha/8 - mask*BIG; P = softmax_k(alpha)
    out = (P @ v) @ Wout                                   [T, D]

Device strategy (one batch per core, no collectives):
  - scores are computed TRANSPOSED, S^T[k,q], so the bias term bs[q,k]*ksum[k,h]
    becomes a per-partition scalar multiply -> one fused DVE scalar_tensor_tensor
    (bias apply + PSUM evacuation + bf16 cast in a single pass).
  - exp without max-subtraction (scores are O(20); fp32 exp range is ample);
    mask enters as an additive -30000 before the exp.
  - context matmul carries a fused ones-column producing softmax denominators.
  - output projection consumes ctx^T directly; host transposes the [D,T] result.
"""

import sys

sys.path.insert(0, "/opt/trn_rl_repo")

import ml_dtypes
import numpy as np

import concourse.bass as bass
import concourse.tile as tile
from concourse import bacc, mybir
from concourse.bass_utils import run_bass_kernel_spmd

BF16 = mybir.dt.bfloat16
F32 = mybir.dt.float32
MULT = mybir.AluOpType.mult
ADD = mybir.AluOpType.add
EXP = mybir.ActivationFunctionType.Exp

B, T, D, H, A = 8, 1024, 1024, 16, 64
HA = H * A
P = 128
KD = D // P      # 8 contraction tiles over D
KT = T // P      # 8 tiles over key tokens
NQ = 2           # query-token 512-chunks
NC_ = 512        # free-dim chunk
MASK_NEG = -30000.0

_CACHED_NC = None


def _build_nc():
    nc = bacc.Bacc("TRN2", target_bir_lowering=False, debug=False, num_devices=8)

    xT = nc.dram_tensor("xT", [D, T], BF16, kind="ExternalInput")     # states^T
    yT = nc.dram_tensor("yT", [D, T], BF16, kind="ExternalInput")     # key_states^T
    wq = nc.dram_tensor("wq", [D, HA], BF16, kind="ExternalInput")
    wk = nc.dram_tensor("wk", [D, HA], BF16, kind="ExternalInput")
    wv = nc.dram_tensor("wv", [D, HA], BF16, kind="ExternalInput")
    wks = nc.dram_tensor("wks", [D, H], BF16, kind="ExternalInput")   # Wk.sum(A)
    wo = nc.dram_tensor("wo", [HA, D], BF16, kind="ExternalInput")
    bsm = nc.dram_tensor("bsm", [T, T], BF16, kind="ExternalInput")   # bias scatter ^T
    mneg = nc.dram_tensor("mneg", [T, T], BF16, kind="ExternalInput")  # -30000*mask ^T
    out = nc.dram_tensor("out", [D, T], F32, kind="ExternalOutput")   # result ^T

    with tile.TileContext(nc) as tc:
        with tc.tile_pool(name="persist", bufs=1) as pp, \
             tc.tile_pool(name="dscr", bufs=1, space="DRAM") as dpool:
            qT = [pp.tile([P, T], BF16, tag=f"qT{i}", name=f"qT{i}") for i in range(KT)]
            kTt = [pp.tile([P, T], BF16, tag=f"kT{i}", name=f"kT{i}") for i in range(KT)]
            v_sb = [pp.tile([P, H, A + 1], BF16, tag=f"v{i}", name=f"v{i}") for i in range(KT)]
            ksum = pp.tile([P, KT * H], F32, tag="ksum", name="ksum")   # col = kt*16 + h
            ctxT = [pp.tile([P, T], BF16, tag=f"ctx{i}", name=f"ctx{i}") for i in range(KT)]
            wo_sb = [pp.tile([P, D], BF16, tag=f"wo{i}", name=f"wo{i}") for i in range(KT)]
            rs = pp.tile([2 * H, NC_], F32, tag="rs", name="rs")      # row = h*2 + n
            rsr = pp.tile([2 * H, NC_], F32, tag="rsr", name="rsr")
            scr = dpool.tile([2 * H, NC_], F32, name="scr")

            for i in range(KT):
                nc.sync.dma_start(wo_sb[i][:], wo.ap()[i * P:(i + 1) * P, :])

            # ---------------- phase A: projections ----------------
            with tc.tile_pool(name="pa", bufs=1) as pa, \
                 tc.tile_pool(name="paps", bufs=4, space="PSUM") as paps:
                xTs = [pa.tile([P, T], BF16, tag=f"xTs{i}", name=f"xTs{i}") for i in range(KD)]
                yTs = [pa.tile([P, T], BF16, tag=f"yTs{i}", name=f"yTs{i}") for i in range(KD)]
                wqs = [pa.tile([P, HA], BF16, tag=f"wqs{i}", name=f"wqs{i}") for i in range(KD)]
                wkt = [pa.tile([P, HA], BF16, tag=f"wkt{i}", name=f"wkt{i}") for i in range(KD)]
                wvt = [pa.tile([P, HA], BF16, tag=f"wvt{i}", name=f"wvt{i}") for i in range(KD)]
                wkss = pa.tile([P, KD * H], BF16, tag="wkss", name="wkss")
                for i in range(KD):
                    sl = slice(i * P, (i + 1) * P)
                    nc.sync.dma_start(xTs[i][:], xT.ap()[sl, :])
                    nc.sync.dma_start(yTs[i][:], yT.ap()[sl, :])
                    nc.sync.dma_start(wqs[i][:], wq.ap()[sl, :])
                    nc.sync.dma_start(wkt[i][:], wk.ap()[sl, :])
                    nc.sync.dma_start(wvt[i][:], wv.ap()[sl, :])
                    nc.sync.dma_start(wkss[:, i * H:(i + 1) * H], wks.ap()[sl, :])

                # qT[m] = (Wq[:, m-tile]^T @ statesT),  kT likewise
                for m in range(KT):
                    msl = slice(m * P, (m + 1) * P)
                    for n in range(NQ):
                        nsl = slice(n * NC_, (n + 1) * NC_)
                        ps = paps.tile([P, NC_], F32, tag="paps", name="paps")
                        for kd in range(KD):
                            nc.tensor.matmul(ps[:], wqs[kd][:, msl],
                                             xTs[kd][:, nsl],
                                             start=(kd == 0), stop=(kd == KD - 1))
                        nc.scalar.copy(qT[m][:, nsl], ps[:])
                for m in range(KT):
                    msl = slice(m * P, (m + 1) * P)
                    for n in range(NQ):
                        nsl = slice(n * NC_, (n + 1) * NC_)
                        ps = paps.tile([P, NC_], F32, tag="paps", name="paps")
                        for kd in range(KD):
                            nc.tensor.matmul(ps[:], wkt[kd][:, msl],
                                             yTs[kd][:, nsl],
                                             start=(kd == 0), stop=(kd == KD - 1))
                        nc.scalar.copy(kTt[m][:, nsl], ps[:])

                # ksum[m-tile, h] = key @ Wk.sum(A)
                for m in range(KT):
                    msl = slice(m * P, (m + 1) * P)
                    ps = paps.tile([P, NC_], F32, tag="paps", name="paps")
                    for kd in range(KD):
                        nc.tensor.matmul(ps[:, 0:H], yTs[kd][:, msl],
                                         wkss[:, kd * H:(kd + 1) * H],
                                         start=(kd == 0), stop=(kd == KD - 1))
                    nc.vector.tensor_copy(ksum[:, m * H:(m + 1) * H], ps[:, 0:H])

                # v[m-tile] = key @ Wv, laid out [P, H, 65] with ones in col 64
                for m in range(KT):
                    msl = slice(m * P, (m + 1) * P)
                    nc.gpsimd.memset(v_sb[m][:, :, A:A + 1], 1.0)
                    for n in range(NQ):
                        nsl = slice(n * NC_, (n + 1) * NC_)
                        ps = paps.tile([P, NC_], F32, tag="paps", name="paps")
                        for kd in range(KD):
                            nc.tensor.matmul(ps[:], yTs[kd][:, msl],
                                             wvt[kd][:, nsl],
                                             start=(kd == 0), stop=(kd == KD - 1))
                        nc.scalar.copy(
                            v_sb[m][:, n * (H // 2):(n + 1) * (H // 2), 0:A],
                            ps[:].rearrange("p (h a) -> p h a", a=A))

            # ---------------- phase B: attention ----------------
            with tc.tile_pool(name="pb", bufs=1) as pb, \
                 tc.tile_pool(name="ptmp", bufs=2) as ptmp, \
                 tc.tile_pool(name="prst", bufs=3) as prst, \
                 tc.tile_pool(name="pblk", bufs=4) as pblk, \
                 tc.tile_pool(name="sps", bufs=6, space="PSUM") as spsum, \
                 tc.tile_pool(name="cps", bufs=2, space="PSUM") as cpsum:
                bsm_sb = [pb.tile([P, T], BF16, tag=f"bsm{i}", name=f"bsm{i}")
                          for i in range(KT)]
                # mneg laid out kt-major per q-chunk: [P, KT, NC_]
                mneg_n = [pb.tile([P, KT, NC_], BF16, tag=f"mnegn{n}",
                                  name=f"mnegn{n}") for n in range(NQ)]
                for i in range(KT):
                    sl = slice(i * P, (i + 1) * P)
                    nc.sync.dma_start(bsm_sb[i][:], bsm.ap()[sl, :])
                    for n in range(NQ):
                        nc.sync.dma_start(
                            mneg_n[n][:, i, :],
                            mneg.ap()[sl, n * NC_:(n + 1) * NC_])

                GPS_KT = 6  # kt >= GPS_KT mask-adds go to GPSIMD

                def emit_scores(hp, n):
                    nsl = slice(n * NC_, (n + 1) * NC_)
                    pblks = []
                    s1b = [ptmp.tile([P, KT, NC_], BF16, tag="s1", name="s1")
                           for _ in range(2)]
                    for kt in range(KT):
                        for hi in range(2):
                            h = 2 * hp + hi
                            roff = hi * A
                            sps = spsum.tile([P, NC_], F32, tag="sps", name="sps")
                            nc.tensor.matmul(
                                sps[:],
                                kTt[hp][roff:roff + A, kt * P:(kt + 1) * P],
                                qT[hp][roff:roff + A, nsl],
                                start=True, stop=True)
                            nc.vector.scalar_tensor_tensor(
                                s1b[hi][:, kt, :], bsm_sb[kt][:, nsl],
                                ksum[:, kt * H + h:kt * H + h + 1],
                                sps[:], op0=MULT, op1=ADD)
                    for hi in range(2):
                        pbk = pblk.tile([P, KT, NC_], BF16, tag="Pblk",
                                        name="Pblk")
                        nc.vector.tensor_tensor(
                            pbk[:, 0:GPS_KT, :], s1b[hi][:, 0:GPS_KT, :],
                            mneg_n[n][:, 0:GPS_KT, :], op=ADD)
                        nc.gpsimd.tensor_tensor(
                            pbk[:, GPS_KT:KT, :], s1b[hi][:, GPS_KT:KT, :],
                            mneg_n[n][:, GPS_KT:KT, :], op=ADD)
                        nc.scalar.activation(pbk[:], pbk[:], EXP, scale=0.125)
                        pblks.append(pbk)
                    return pblks

                def emit_ctx(hp, n, pblks):
                    nsl = slice(n * NC_, (n + 1) * NC_)
                    for hi in range(2):
                        h = 2 * hp + hi
                        roff = hi * A
                        cps = cpsum.tile([A + 1, NC_], F32, tag="cps",
                                         name="cps")
                        for kt in range(KT):
                            nc.tensor.matmul(
                                cps[:], v_sb[kt][:, h, :], pblks[hi][:, kt, :],
                                start=(kt == 0), stop=(kt == KT - 1))
                        r = h * 2 + n
                        rstage = prst.tile([1, NC_], F32, tag="rstage",
                                           name="rstage")
                        nc.scalar.copy(rstage[:], cps[A:A + 1, :])
                        nc.sync.dma_start(rs[r:r + 1, :], rstage[:])
                        nc.scalar.copy(ctxT[hp][roff:roff + A, nsl],
                                       cps[0:A, :])

                iters = [(hp, n) for hp in range(H // 2) for n in range(NQ)]
                prev = None
                for hp, n in iters:
                    pblks = emit_scores(hp, n)
                    if prev is not None:
                        emit_ctx(*prev)
                    prev = (hp, n, pblks)
                emit_ctx(*prev)

                # normalization: batched reciprocal, broadcast via DRAM bounce
                nc.vector.reciprocal(rsr[:], rs[:])
                nc.sync.dma_start(scr[:], rsr[:])
                with tc.tile_pool(name="rbp", bufs=3) as rbp:
                    for hp in range(H // 2):
                        for n in range(NQ):
                            nsl = slice(n * NC_, (n + 1) * NC_)
                            r0 = (2 * hp) * 2 + n
                            r1 = (2 * hp + 1) * 2 + n
                            rb = rbp.tile([P, NC_], F32, tag="rb", name="rb")
                            src0 = bass.AP(scr[:].tensor,
                                           scr[:].offset + r0 * NC_,
                                           [[0, A], [1, NC_]])
                            src1 = bass.AP(scr[:].tensor,
                                           scr[:].offset + r1 * NC_,
                                           [[0, A], [1, NC_]])
                            nc.sync.dma_start(rb[0:A, :], src0)
                            nc.sync.dma_start(rb[A:P, :], src1)
                            nc.vector.tensor_tensor(
                                ctxT[hp][:, nsl],
                                ctxT[hp][:, nsl], rb[:], op=MULT)

            # ---------------- phase C: output projection ----------------
            with tc.tile_pool(name="po", bufs=3) as po, \
                 tc.tile_pool(name="ops", bufs=4, space="PSUM") as opsum:
                for m in range(KD):
                    msl = slice(m * P, (m + 1) * P)
                    for n in range(NQ):
                        nsl = slice(n * NC_, (n + 1) * NC_)
                        ps = opsum.tile([P, NC_], F32, tag="ops", name="ops")
                        for kt in range(KT):
                            nc.tensor.matmul(ps[:], wo_sb[kt][:, msl],
                                             ctxT[kt][:, nsl],
                                             start=(kt == 0), stop=(kt == KT - 1))
                        osb = po.tile([P, NC_], F32, tag="osb", name="osb")
                        nc.scalar.copy(osb[:], ps[:])
                        nc.sync.dma_start(out.ap()[msl, nsl], osb[:])

    nc.compile()
    return nc


def _get_nc():
    global _CACHED_NC
    if _CACHED_NC is None:
        _CACHED_NC = _build_nc()
    return _CACHED_NC


def _prep_inputs(states, key_states, masks, attention_bias, Wq, Wk, Wv, Wout,
                 bias_embs, bias_scalar):
    bf = ml_dtypes.bfloat16
    states = np.asarray(states, dtype=np.float32)
    key_states = np.asarray(key_states, dtype=np.float32)
    masks = np.asarray(masks, dtype=np.float32)
    ab = np.asarray(attention_bias)
    Wq = np.asarray(Wq, dtype=np.float32).reshape(D, HA)
    Wk3 = np.asarray(Wk, dtype=np.float32)
    Wv = np.asarray(Wv, dtype=np.float32).reshape(D, HA)
    Wout = np.asarray(Wout, dtype=np.float32).reshape(HA, D)
    bias_embs = np.asarray(bias_embs, dtype=np.float32)
    bias_scalar = np.asarray(bias_scalar, dtype=np.float32)

    bvals = (bias_embs[ab[:, 0]] @ bias_scalar)[:, 0]          # [E]
    wq_b = np.ascontiguousarray(Wq).astype(bf)
    wk_b = np.ascontiguousarray(Wk3.reshape(D, HA)).astype(bf)
    wv_b = np.ascontiguousarray(Wv).astype(bf)
    wks_b = np.ascontiguousarray(Wk3.sum(axis=2)).astype(bf)   # [D, H]
    wo_b = np.ascontiguousarray(Wout).astype(bf)

    in_maps = []
    for b in range(B):
        bs = np.zeros((T, T), dtype=np.float32)
        sel = ab[:, 1] == b
        bs[ab[sel, 2], ab[sel, 3]] = bvals[sel]                # last write wins
        in_maps.append({
            "xT": np.ascontiguousarray(states[b].T).astype(bf),
            "yT": np.ascontiguousarray(key_states[b].T).astype(bf),
            "wq": wq_b, "wk": wk_b, "wv": wv_b, "wks": wks_b, "wo": wo_b,
            "bsm": np.ascontiguousarray(bs.T).astype(bf),
            "mneg": np.ascontiguousarray(masks[b].T * MASK_NEG).astype(bf),
        })
    return in_maps


def kernel(**inputs) -> np.ndarray:
    nc = _get_nc()
    in_maps = _prep_inputs(**inputs)
    res = run_bass_kernel_spmd(nc, in_maps, core_ids=list(range(8)))
    out = np.empty((B, T, D), dtype=np.float32)
    for b in range(B):
        out[b] = res.results[b]["out"].T
    return out
